# revision 14
# baseline (speedup 1.0000x reference)
"""CAM (channel attention) module kernel for Trainium2, 8-core data-parallel.

Reference computation (per batch b, channel c):
    v = x[b,c]                         # (P=3, HW=4096)
    energy = v @ v.T                   # (3,3) Gram matrix
    en = rowmax(energy) - energy
    att = softmax(en, axis=-1)
    out = att @ v                      # (3, 4096)
    y[b,c] = gamma * out + x[b,c]

Sharding: batch dim (B=8) across the 8 NeuronCores; no cross-core comms.
Per core: channels on SBUF partitions (2 groups of 128), free dim = P*HW.
  - Gram diagonal:  ScalarE Square activation with accum_out (free-axis sum)
  - Gram off-diag:  VectorE scalar_tensor_tensor with accum_out (fused
                    elementwise product + free-axis sum in one op)
  - softmax:        tiny [128, 3x3] ops on VectorE/ScalarE
  - mix (att @ v):  TensorEngine. Coeffs C = gamma*att + I are expanded into
                    per-(i,j) DIAGONAL 128x128 matrices D_ij = diag(C[:,i,j])
                    (one ScalarE scaled-copy of a precomputed identity each).
                    Then y_i = sum_j D_ij.T @ v_j as fp32r matmuls (1 cyc/row)
                    accumulating in PSUM; out-DMA streams straight PSUM->HBM.
This moves all 6 VectorE MAC passes + 3 ScalarE premult passes per group off
the elementwise engines onto the otherwise-idle PE array, leaving the kernel
HBM-bound (25.2 MB/core). VectorE keeps only the 3 cross-gram reductions.
"""

import numpy as np

import concourse.bacc as bacc
import concourse.mybir as mybir
import concourse.tile as tile
from concourse.bass_utils import run_bass_kernel_spmd

B, C, P, H, W = 8, 256, 3, 64, 64
HW = H * W
N_CORES = 8
PARTS = 128
MMCHUNK = 512  # moving-dim elements per matmul instr = one 2KB PSUM bank

F32 = mybir.dt.float32
F32R = mybir.dt.float32r
Alu = mybir.AluOpType
Act = mybir.ActivationFunctionType


def build_nc(C_=C, HW_=HW, repeat=1, split_load=True):
    """Build the per-core Bass program. Each core sees x:(C_,P,HW_), gamma:(1,1).

    repeat>1 re-runs the whole computation (same I/O, idempotent) that many
    times in one program — used by test.py to time the kernel by slope.
    split_load: 3 per-path load DMAs (measured faster than one big DMA).
    """
    assert C_ % PARTS == 0 and HW_ % MMCHUNK == 0
    n_groups = C_ // PARTS
    n_chunks = HW_ // MMCHUNK

    nc = bacc.Bacc("TRN2", target_bir_lowering=False, debug=False)

    # x is declared float32r (same 4-byte fp32 bits; np dtype maps to float32)
    # so the load DMA is cast-free and the PE accepts v as an fp32r operand.
    x_d = nc.dram_tensor("x", [C_, P, HW_], F32R, kind="ExternalInput")
    g_d = nc.dram_tensor("gamma", [1, 1], F32, kind="ExternalInput")
    y_d = nc.dram_tensor("y", [C_, P, HW_], F32, kind="ExternalOutput")

    with tile.TileContext(nc) as tc:
        with (
            tc.tile_pool(name="consts", bufs=1) as consts,
            tc.tile_pool(name="vpool", bufs=2) as vpool,
            tc.tile_pool(name="scratch", bufs=1) as scratch,
            tc.tile_pool(name="smalls", bufs=2) as smalls,
            tc.tile_pool(name="dpool", bufs=2) as dpool,
            tc.psum_pool(name="pp", bufs=1) as pp,
        ):
            # --- constants (once) ---
            gsb = consts.tile([1, 1], F32)
            nc.sync.dma_start(gsb[:], g_d[:])
            gamma_bc = consts.tile([PARTS, 1], F32)
            nc.gpsimd.partition_broadcast(gamma_bc[:], gsb[:])

            ident = consts.tile([PARTS, 9], F32)
            nc.vector.memset(ident[:], 0.0)
            for i in range(P):
                nc.vector.memset(ident[:, 4 * i : 4 * i + 1], 1.0)

            # 128x128 identity for expanding per-channel coeffs to diagonal
            # matmul weights: ident128[p,f] = (p == f)
            ident128 = consts.tile([PARTS, PARTS], F32)
            nc.gpsimd.memset(ident128[:], 1.0)
            nc.gpsimd.affine_select(
                ident128[:], ident128[:], pattern=[[-1, PARTS]],
                compare_op=Alu.is_equal, fill=0.0,
                base=0, channel_multiplier=1,
            )

            for g in range(n_groups * repeat):
                g = g % n_groups
                cs = slice(g * PARTS, (g + 1) * PARTS)

                # --- load group: 3 path-split DMAs so compute starts early ---
                # Tile dtype is float32r so the PE matmul verifier accepts it
                # as a moving operand; ACT/DVE gram ops read it via a
                # bitcast-to-f32 view (same bits).
                vr = vpool.tile([PARTS, P, HW_], F32R)
                if split_load:
                    for i in range(P):
                        nc.sync.dma_start(vr[:, i, :], x_d[cs, i, :])
                else:
                    nc.sync.dma_start(vr[:], x_d[cs, :, :])

                # --- phase 1: per-channel 3x3 Gram matrix over HW ---
                E = smalls.tile([PARTS, 9], F32)
                # per-engine scratch tags (write-only garbage): one slot per
                # engine keeps ACT and DVE from serializing on shared slots
                for i in range(P):  # diagonal terms on ScalarE
                    scr = scratch.tile([PARTS, HW_], F32, tag="scr_act", bufs=1)
                    nc.scalar.activation(
                        scr[:], vr[:, i, :].bitcast(F32), Act.Square,
                        accum_out=E[:, 4 * i : 4 * i + 1],
                    )
                for i, j, col in ((0, 1, 1), (1, 2, 5), (0, 2, 2)):  # cross on VectorE
                    # NOTE: tensor_tensor_reduce wedges the exec unit on this
                    # runtime (NRT_EXEC_UNIT_UNRECOVERABLE); scalar_tensor_tensor
                    # with accum_out is the same fused mult+reduce via the
                    # standard TensorScalarPtr opcode and works. GPSIMD can't
                    # take one either: Pool stst+accum_out dies in a walrus
                    # backend pass.
                    scr = scratch.tile([PARTS, HW_], F32, tag="scr_dve", bufs=1)
                    nc.vector.scalar_tensor_tensor(
                        scr[:], vr[:, i, :].bitcast(F32), 1.0,
                        vr[:, j, :].bitcast(F32),
                        op0=Alu.bypass, op1=Alu.mult,
                        accum_out=E[:, col : col + 1],
                    )
                # mirror symmetric entries: (1,0)<-(0,1), (2,1)<-(1,2), (2,0)<-(0,2)
                # on ScalarE (Copy) to keep VectorE's instruction count down
                for src, dst in ((1, 3), (5, 7), (2, 6)):
                    nc.scalar.copy(E[:, dst : dst + 1], E[:, src : src + 1])

                # --- softmax over rows of the 3x3, coeffs = gamma*att + I ---
                E3 = E.rearrange("p (i j) -> p i j", j=P)
                M = smalls.tile([PARTS, P, 1], F32)
                # reference computes softmax(rowmax - E); softmax is shift
                # invariant, so use (rowmin - E) instead: exponents stay <= 0
                # (numerically stable without a second max pass).
                nc.vector.tensor_reduce(M[:], E3, axis=mybir.AxisListType.X, op=Alu.min)
                # EX[i,j] = exp(rowmin_i - E[i,j]) via per-row ScalarE Exp with
                # scale=-1, bias=rowmin_i (per-partition AP) — no DVE subtract
                EX = smalls.tile([PARTS, P, P], F32)
                for i in range(P):
                    nc.scalar.activation(
                        EX[:, i, :], E3[:, i, :], Act.Exp,
                        scale=-1.0, bias=M[:, i, 0:1],
                    )
                S = smalls.tile([PARTS, P, 1], F32)
                nc.vector.tensor_reduce(S[:], EX[:], axis=mybir.AxisListType.X, op=Alu.add)
                R = smalls.tile([PARTS, P, 1], F32)
                nc.vector.reciprocal(R[:], S[:])
                A = smalls.tile([PARTS, P, P], F32)
                nc.vector.tensor_mul(A[:], EX[:], R[:].broadcast_to([PARTS, P, P]))
                Cf = smalls.tile([PARTS, 9], F32)
                nc.vector.scalar_tensor_tensor(
                    Cf[:].rearrange("p (i j) -> p i j", j=P), A[:], gamma_bc[:, 0:1],
                    ident[:].rearrange("p (i j) -> p i j", j=P),
                    op0=Alu.mult, op1=Alu.add,
                )

                # --- phase 2 weights: D_ij = diag(Cf[:, 3i+j]) via ScalarE
                # scaled copy of ident128 (scale = per-partition AP) ---
                Dw = dpool.tile([PARTS, 9, PARTS], F32R, tag="Dw")
                for k in range(9):
                    nc.scalar.activation(
                        Dw[:, k, :], ident128[:], Act.Copy,
                        scale=Cf[:, k : k + 1],
                    )

                # --- phase 2: y_i = sum_j Cf[i,j]*v_j on the PE array ---
                # lhsT = D_ij [K=128, M=128]: out[m,n] = sum_k D[k,m] v_j[k,n]
                #      = Cf[m,3i+j] * v_j[m,n];  PSUM accumulates over j.
                # fp32r runs the moving operand at 1 row/cycle (N=512 >= 256).
                # PSUM granularity: quarter-rows [128, 1024] (2 banks, 4 bufs
                # rotating); DMA can't read PSUM, so each quarter bounces
                # through SBUF, round-robining the copy across ACT/DVE/Pool
                # so no single engine sits on the PE->DMA critical path.
                quart = HW_ // 4
                for i in range(P):
                    for h in range(4):
                        ps = pp.tile([PARTS, quart], F32, tag="ps", bufs=4)
                        for cch in range(quart // MMCHUNK):
                            sl = slice(cch * MMCHUNK, (cch + 1) * MMCHUNK)
                            gl = slice(h * quart + cch * MMCHUNK,
                                       h * quart + (cch + 1) * MMCHUNK)
                            for j in range(P):
                                nc.tensor.matmul(
                                    ps[:, sl],
                                    Dw[:, 3 * i + j, :],
                                    vr[:, j, gl],
                                    start=(j == 0), stop=(j == P - 1),
                                )
                        t = vpool.tile([PARTS, quart], F32, tag="t", bufs=4)
                        # GPSIMD cannot read PSUM (walrus verifier rejects it),
                        # so the drain alternates ACT/DVE only.
                        if (4 * i + h) % 2 == 0:
                            nc.scalar.copy(t[:], ps[:])
                        else:
                            nc.vector.tensor_copy(t[:], ps[:])
                        nc.sync.dma_start(
                            y_d[cs, i, h * quart : (h + 1) * quart], t[:]
                        )

    nc.compile()
    return nc


_NC_CACHE = {}


def _get_nc(C_=C, HW_=HW):
    key = (C_, HW_)
    if key not in _NC_CACHE:
        _NC_CACHE[key] = build_nc(C_, HW_)
    return _NC_CACHE[key]


def run_full(x: np.ndarray, gamma: np.ndarray, **runner_kwargs):
    """Run on all 8 cores; returns the raw BassKernelResults."""
    x = np.asarray(x, dtype=np.float32)
    gamma = np.asarray(gamma, dtype=np.float32)
    assert x.shape == (B, C, P, H, W), x.shape

    nc = _get_nc()
    in_maps = [
        {
            "x": np.ascontiguousarray(x[k]).reshape(C, P, HW),
            "gamma": gamma.reshape(1, 1),
        }
        for k in range(N_CORES)
    ]
    return run_bass_kernel_spmd(
        nc, in_maps, core_ids=list(range(N_CORES)), **runner_kwargs
    )


def kernel(x: np.ndarray, gamma: np.ndarray) -> np.ndarray:
    res = run_full(x, gamma)
    y = np.stack([res.results[k]["y"] for k in range(N_CORES)])
    return y.reshape(B, C, P, H, W)
